# revision 5
# baseline (speedup 1.0000x reference)
"""GIN (4-layer) message-passing kernel for Trainium2, 8-core SPMD.

Strategy:
  - Shard nodes/edges by destination-node ownership (12500 nodes per core).
  - Aggregation (segment_sum of x[src] by dst) is computed on the TensorEngine
    as a sequence of one-hot segment matmuls: gather 128-edge tiles of source
    features via SWDGE dma_gather (int16 indices, 4 source windows of 32768
    rows), build a one-hot [edges x segs] matrix on the VectorEngine with an
    is_equal compare against an iota, then matmul E^T @ S -> agg_T in PSUM.
  - BN is folded into the MLP weights host-side; the MLP runs feature-major
    (features on partitions) so biases are per-partition ACT operands.
  - After each layer, the new node features are transposed back to node-major
    (PE transpose) and AllGather'ed across the 8 cores to rebuild the full
    feature table for the next layer's gathers.
  - Graph pooling = one-hot matmul against sorted batch ids, AllReduce, then a
    small replicated MLP head with log_softmax.

The per-(src-window, dst-window) edge-group sizes are padded to the max across
cores (and to multiples of 128) so all 8 cores run one identical program.
"""

import math
import numpy as np

# ---- problem constants (hardcoded; kernel.py must be self-contained) ----
N_NODES = 100000
N_EDGES = 1600000
C = 128            # feature channels
HID = 128
N_LAYERS = 4
N_GRAPHS = 512
N_OUT = 10
BN_EPS = 1e-5
N_CORES = 8

# ---- kernel configuration ----
class Cfg:
    def __init__(self, n_nodes=N_NODES, n_graphs=N_GRAPHS, win=32768,
                 dwin=128, gchunk=8192, node_chunk=512, table_f16=True):
        self.n_nodes = n_nodes
        self.n_graphs = n_graphs
        self.npc = n_nodes // N_CORES          # nodes per core
        self.win = win                          # src window (int16 gather)
        self.nsw = -(-n_nodes // win)           # number of src windows
        self.dwin = dwin                        # dst segment window (psum)
        self.ndw = -(-self.npc // dwin)         # dst windows per core
        self.gchunk = gchunk                    # gather chunk (edges/call)
        self.node_chunk = node_chunk            # MLP node chunk
        self.table_f16 = table_f16
        # MLP chunk list: full chunks + remainder
        ch = []
        off = 0
        while off < self.npc:
            n = min(node_chunk, self.npc - off)
            ch.append((off, n))
            off += n
        self.mlp_chunks = ch
        self.npt = -(-self.npc // 128)          # pooling node tiles


DEFAULT_CFG = Cfg()

_BUILD_CACHE = {}


# =========================================================================
# host-side preprocessing
# =========================================================================

def _fold_weights(ins):
    f32 = np.float32
    s1 = (np.asarray(ins["bn1_g"], f32)
          / np.sqrt(np.asarray(ins["bn1_v"], f32) + BN_EPS))
    w1f = np.asarray(ins["w1"], f32) * s1[:, None, :]
    b1f = (np.asarray(ins["b1"], f32) * s1
           + np.asarray(ins["bn1_b"], f32) - np.asarray(ins["bn1_m"], f32) * s1)
    s2 = (np.asarray(ins["bn2_g"], f32)
          / np.sqrt(np.asarray(ins["bn2_v"], f32) + BN_EPS))
    w2f = np.asarray(ins["w2"], f32) * s2[:, None, :]
    b2f = (np.asarray(ins["b2"], f32) * s2
           + np.asarray(ins["bn2_b"], f32) - np.asarray(ins["bn2_m"], f32) * s2)
    s3 = (np.asarray(ins["bn3_g"], f32)
          / np.sqrt(np.asarray(ins["bn3_v"], f32) + BN_EPS))
    lin1f = np.asarray(ins["lin1_w"], f32) * s3[None, :]
    lin1b = (np.asarray(ins["lin1_b"], f32) * s3
             + np.asarray(ins["bn3_b"], f32) - np.asarray(ins["bn3_m"], f32) * s3)
    nl = w1f.shape[0]
    h2 = w1f.shape[2]  # 2*HID
    hid = w2f.shape[2]
    return dict(
        w1f=np.ascontiguousarray(w1f),                                  # [L,C,2H]
        b1fT=np.ascontiguousarray(
            b1f.reshape(nl, h2 // 128, 128).transpose(2, 0, 1).reshape(128, -1)),
        w2f=np.ascontiguousarray(w2f.reshape(nl, h2 // 128, 128, hid)),  # [L,2,128,H]
        b2fT=np.ascontiguousarray(b2f.T),                                # [H,L]
        lin1f=np.ascontiguousarray(lin1f),                               # [C,H]
        lin1bT=np.ascontiguousarray(lin1b[:, None]),                     # [H,1]
        lin2=np.ascontiguousarray(np.asarray(ins["lin2_w"], f32)),       # [H,O]
        lin2b=np.ascontiguousarray(np.asarray(ins["lin2_b"], f32)[None, :]),
        eps=[float(e) for e in np.asarray(ins["gin_eps"], f32)],
    )


def _preprocess(ins, cfg: Cfg):
    tdt = np.float16 if cfg.table_f16 else np.float32
    x = np.asarray(ins["x"], np.float32)
    ei = np.asarray(ins["edge_index"])
    src = ei[0].astype(np.int64)
    dst = ei[1].astype(np.int64)
    batch = np.asarray(ins["batch"]).astype(np.int64)
    E = src.shape[0]

    npc, win, nsw, dwin, ndw = cfg.npc, cfg.win, cfg.nsw, cfg.dwin, cfg.ndw

    core = dst // npc
    dloc = dst - core * npc
    w_of = src // win
    dw_of = dloc // dwin
    key = (core * nsw + w_of) * ndw + dw_of

    counts = np.bincount(key, minlength=N_CORES * nsw * ndw).reshape(
        N_CORES, nsw, ndw)
    gmax_t = -(-counts.max(axis=0) // 128)            # tiles per (w,dw)
    nt_sec = gmax_t.sum(axis=1)                        # tiles per section
    NT = int(gmax_t.sum())
    NLIST = NT * 128

    # global tile offset of each (w, dw) group
    gt_off = np.zeros((nsw, ndw), np.int64)
    cum = 0
    for w in range(nsw):
        for dw in range(ndw):
            gt_off[w, dw] = cum
            cum += gmax_t[w, dw]

    # stable order by (core, w, dw); rank within group
    order = np.argsort(key, kind="stable")
    sk = key[order]
    first = np.r_[True, sk[1:] != sk[:-1]]
    gfp = np.where(first)[0]
    runs = np.diff(np.r_[gfp, E])
    rank = np.arange(E) - np.repeat(gfp, runs)

    gt_off_flat = gt_off.reshape(-1)                   # per (w,dw)
    wd_key = (w_of * ndw + dw_of)[order]
    pos = gt_off_flat[wd_key] * 128 + rank             # position in core's list

    src_rel = (src - w_of * win)[order].astype(np.int16)
    seg_rel = (dloc - dw_of * dwin)[order].astype(np.int64)
    core_s = core[order]

    # gather-call chunking per section (chunks of <= gchunk edges, 128-mult)
    gct = cfg.gchunk // 128
    chunks = []                    # (w, tile_start_global, ntiles)
    sec_t0 = 0
    for w in range(nsw):
        t = 0
        while t < nt_sec[w]:
            n = min(gct, nt_sec[w] - t)
            chunks.append((w, int(sec_t0 + t), int(n)))
            t += n
        sec_t0 += nt_sec[w]

    first_w = np.full(ndw, -1, np.int64)
    for dw in range(ndw):
        for w in range(nsw):
            if gmax_t[w, dw] > 0:
                first_w[dw] = w
                break

    per_core = []
    for c in range(N_CORES):
        m = core_s == c
        lst = np.zeros(NLIST, np.int16)
        seg = np.full(NLIST, -1.0, np.float32)
        lst[pos[m]] = src_rel[m]
        seg[pos[m]] = seg_rel[m]
        # wrap indices per gather call: entry i of a call at [i%16, i//16]
        gidxw = np.zeros((16, NLIST // 16), np.int16)
        for (w, t0, nt) in chunks:
            a, b = t0 * 128, (t0 + nt) * 128
            gidxw[:, t0 * 8: t0 * 8 + nt * 8] = lst[a:b].reshape(-1, 16).T
        gidxw = np.tile(gidxw, (8, 1))
        segcols = np.ascontiguousarray(seg.reshape(NT, 128).T.astype(tdt))

        # pooling seg (batch value per own node), pad -1
        bslab = batch[c * npc:(c + 1) * npc].astype(np.float32)
        pseg = np.full(cfg.npt * 128, -1.0, np.float32)
        pseg[:npc] = bslab
        psegcols = np.ascontiguousarray(
            pseg.reshape(cfg.npt, 128).T.astype(tdt))

        xT0 = np.ascontiguousarray(x[c * npc:(c + 1) * npc].T.astype(np.float32))
        per_core.append(dict(gidx=np.ascontiguousarray(gidxw),
                             segrel=segcols, psegrel=psegcols, xT0=xT0))

    wf = _fold_weights(ins)
    iota_seg = np.ascontiguousarray(
        np.broadcast_to(np.arange(cfg.dwin, dtype=tdt), (128, cfg.dwin)))
    iota_g = np.ascontiguousarray(
        np.broadcast_to(np.arange(cfg.n_graphs, dtype=tdt),
                        (128, cfg.n_graphs)))
    shared = dict(
        xtab0=np.ascontiguousarray(x.astype(tdt)),
        w1f=wf["w1f"], b1fT=wf["b1fT"], w2f=wf["w2f"], b2fT=wf["b2fT"],
        lin1f=wf["lin1f"], lin1bT=wf["lin1bT"], lin2=wf["lin2"],
        lin2b=wf["lin2b"], iota_seg=iota_seg, iota_g=iota_g,
    )

    meta = dict(
        gmax_t=gmax_t, chunks=chunks, first_w=first_w, NT=NT, NLIST=NLIST,
        gt_off=gt_off, eps=wf["eps"],
    )
    return shared, per_core, meta


# =========================================================================
# device program
# =========================================================================

def _build_program(meta, cfg: Cfg):
    import concourse.bacc as bacc
    import concourse.bass as bass
    import concourse.mybir as mybir
    import concourse.tile as tile
    from concourse.masks import make_identity

    f32 = mybir.dt.float32
    i16 = mybir.dt.int16
    tdt = mybir.dt.float16 if cfg.table_f16 else mybir.dt.float32
    Alu = mybir.AluOpType
    Act = mybir.ActivationFunctionType

    NN, NPC, NG = cfg.n_nodes, cfg.npc, cfg.n_graphs
    NSW, WIN, DWIN, NDW = cfg.nsw, cfg.win, cfg.dwin, cfg.ndw
    NT, NLIST = meta["NT"], meta["NLIST"]
    gmax_t, gt_off = meta["gmax_t"], meta["gt_off"]
    chunks, first_w = meta["chunks"], meta["first_w"]
    eps = meta["eps"]
    NPT = cfg.npt
    H2 = 2 * HID

    nc = bacc.Bacc("TRN2", target_bir_lowering=False, debug=False,
                   num_devices=N_CORES)

    # ---- I/O ----
    xtab0 = nc.dram_tensor("xtab0", [NN, C], tdt, kind="ExternalInput")
    gidx_d = nc.dram_tensor("gidx", [128, NLIST // 16], i16,
                            kind="ExternalInput")
    segrel_d = nc.dram_tensor("segrel", [128, NT], tdt, kind="ExternalInput")
    psegrel_d = nc.dram_tensor("psegrel", [128, NPT], tdt,
                               kind="ExternalInput")
    xT0 = nc.dram_tensor("xT0", [C, NPC], f32, kind="ExternalInput")
    w1f_d = nc.dram_tensor("w1f", [N_LAYERS, C, H2], f32,
                           kind="ExternalInput")
    b1f_d = nc.dram_tensor("b1fT", [128, N_LAYERS * 2], f32,
                           kind="ExternalInput")
    w2f_d = nc.dram_tensor("w2f", [N_LAYERS, 2, 128, HID], f32,
                           kind="ExternalInput")
    b2f_d = nc.dram_tensor("b2fT", [HID, N_LAYERS], f32, kind="ExternalInput")
    lin1f_d = nc.dram_tensor("lin1f", [C, HID], f32, kind="ExternalInput")
    lin1b_d = nc.dram_tensor("lin1bT", [HID, 1], f32, kind="ExternalInput")
    lin2_d = nc.dram_tensor("lin2", [HID, N_OUT], f32, kind="ExternalInput")
    lin2b_d = nc.dram_tensor("lin2b", [1, N_OUT], f32, kind="ExternalInput")
    iota_seg_d = nc.dram_tensor("iota_seg", [128, DWIN], tdt,
                                kind="ExternalInput")
    iota_g_d = nc.dram_tensor("iota_g", [128, NG], tdt, kind="ExternalInput")
    out_d = nc.dram_tensor("out", [NG, N_OUT], f32, kind="ExternalOutput")

    # ---- internal DRAM ----
    xtabs = [xtab0] + [nc.dram_tensor(f"xtab{l}", [NN, C], tdt)
                       for l in range(1, N_LAYERS)]
    xTs = [xT0] + [nc.dram_tensor(f"xT{l}", [C, NPC], f32)
                   for l in range(1, N_LAYERS)]
    xowns = [nc.dram_tensor(f"xown{l}", [NPC, C], tdt)
             for l in range(N_LAYERS)]
    pool_in = nc.dram_tensor("pool_in", [128, NG], f32)
    pool_out = nc.dram_tensor("pool_out", [128, NG], f32)

    rg = [list(range(N_CORES))]

    with tile.TileContext(nc) as tc:
        with (
            tc.tile_pool(name="pers", bufs=1) as pers,
            tc.tile_pool(name="gp", bufs=3) as gp,
            tc.tile_pool(name="stp", bufs=4) as stp,
            tc.tile_pool(name="mlp", bufs=2) as mlp,
            tc.tile_pool(name="headp", bufs=2) as headp,
            tc.tile_pool(name="psum_seg", bufs=2, space="PSUM") as psum_seg,
            tc.tile_pool(name="psum_mlp", bufs=3, space="PSUM") as psum_mlp,
            tc.tile_pool(name="psum_tp", bufs=2, space="PSUM") as psum_tp,
        ):
            # ---------- persistent loads ----------
            gidx_sb = pers.tile([128, NLIST // 16], i16)
            nc.sync.dma_start(out=gidx_sb[:], in_=gidx_d[:, :])
            segrel_sb = pers.tile([128, NT], tdt)
            nc.sync.dma_start(out=segrel_sb[:], in_=segrel_d[:, :])
            psegrel_sb = pers.tile([128, NPT], tdt)
            nc.sync.dma_start(out=psegrel_sb[:], in_=psegrel_d[:, :])
            iota_seg_sb = pers.tile([128, DWIN], tdt)
            nc.sync.dma_start(out=iota_seg_sb[:], in_=iota_seg_d[:, :])
            iota_g_sb = pers.tile([128, NG], tdt)
            nc.sync.dma_start(out=iota_g_sb[:], in_=iota_g_d[:, :])
            ident_sb = pers.tile([128, 128], f32)
            make_identity(nc, ident_sb[:])
            ones1_sb = pers.tile([1, 128], f32)
            nc.vector.memset(ones1_sb[:], 1.0)

            w1_sb = pers.tile([128, N_LAYERS * H2], f32)
            for l in range(N_LAYERS):
                nc.sync.dma_start(out=w1_sb[:, l * H2:(l + 1) * H2],
                                  in_=w1f_d[l])
            w2_sb = pers.tile([128, N_LAYERS * 2 * HID], f32)
            for l in range(N_LAYERS):
                for k in range(2):
                    o = (l * 2 + k) * HID
                    nc.sync.dma_start(out=w2_sb[:, o:o + HID],
                                      in_=w2f_d[l, k])
            b1_sb = pers.tile([128, N_LAYERS * 2], f32)
            nc.sync.dma_start(out=b1_sb[:], in_=b1f_d[:, :])
            b2_sb = pers.tile([128, N_LAYERS], f32)
            nc.sync.dma_start(out=b2_sb[:], in_=b2f_d[:, :])
            lin1_sb = pers.tile([128, HID], f32)
            nc.sync.dma_start(out=lin1_sb[:], in_=lin1f_d[:, :])
            lin1b_sb = pers.tile([128, 1], f32)
            nc.sync.dma_start(out=lin1b_sb[:], in_=lin1b_d[:, :])
            lin2_sb = pers.tile([128, N_OUT], f32)
            nc.sync.dma_start(out=lin2_sb[:], in_=lin2_d[:, :])
            lin2b_sb = pers.tile([1, N_OUT], f32)
            nc.sync.dma_start(out=lin2b_sb[:], in_=lin2b_d[:, :])

            aggT = pers.tile([128, NPC], f32)

            # ---------- layers ----------
            for l in range(N_LAYERS):
                tab = xtabs[l]
                # gathers (section-major)
                tile2chunk = {}
                for (w, t0, nt) in chunks:
                    g = gp.tile([128, nt * 128], tdt, tag="gch")
                    lo = w * WIN
                    hi = min((w + 1) * WIN, NN)
                    n = nt * 128
                    nc.gpsimd.dma_gather(
                        out_ap=g[:].rearrange("p (t e) -> p t e", e=C),
                        in_ap=tab[lo:hi, :],
                        idxs_ap=gidx_sb[:, t0 * 8: t0 * 8 + nt * 8],
                        num_idxs=n,
                        num_idxs_reg=n,
                        elem_size=C,
                        single_packet=False,
                    )
                    for i in range(nt):
                        tile2chunk[t0 + i] = (g, i)

                # segment matmuls, section-major, accumulate into aggT
                for w in range(NSW):
                    for dw in range(NDW):
                        ntl = int(gmax_t[w, dw])
                        if ntl == 0:
                            continue
                        t0 = int(gt_off[w, dw])
                        ps = psum_seg.tile([128, DWIN], f32, tag="segp")
                        for i in range(ntl):
                            t = t0 + i
                            g, lt = tile2chunk[t]
                            st = stp.tile([128, DWIN], tdt, tag="st")
                            nc.vector.tensor_tensor(
                                out=st[:],
                                in0=segrel_sb[:, t:t + 1].to_broadcast(
                                    [128, DWIN]),
                                in1=iota_seg_sb[:],
                                op=Alu.is_equal)
                            nc.tensor.matmul(
                                out=ps[:],
                                lhsT=g[:, lt * 128:(lt + 1) * 128],
                                rhs=st[:],
                                start=(i == 0), stop=(i == ntl - 1))
                        lo = dw * DWIN
                        wd = min(DWIN, NPC - lo)
                        if first_w[dw] == w:
                            nc.vector.tensor_copy(out=aggT[:, lo:lo + wd],
                                                  in_=ps[:, :wd])
                        else:
                            nc.vector.tensor_add(out=aggT[:, lo:lo + wd],
                                                 in0=aggT[:, lo:lo + wd],
                                                 in1=ps[:, :wd])
                for dw in range(NDW):
                    if first_w[dw] < 0:
                        lo = dw * DWIN
                        wd = min(DWIN, NPC - lo)
                        nc.vector.memset(aggT[:, lo:lo + wd], 0.0)

                # MLP over node chunks (feature-major)
                scale = 1.0 + eps[l]
                for (c0, nch) in cfg.mlp_chunks:
                    xt = mlp.tile([128, cfg.node_chunk], f32, tag="xt")
                    nc.sync.dma_start(out=xt[:, :nch],
                                      in_=xTs[l][:, c0:c0 + nch])
                    z = mlp.tile([128, cfg.node_chunk], f32, tag="z")
                    nc.vector.tensor_scalar(
                        out=z[:, :nch], in0=xt[:, :nch],
                        scalar1=scale, scalar2=None, op0=Alu.mult)
                    nc.vector.tensor_add(out=z[:, :nch], in0=z[:, :nch],
                                         in1=aggT[:, c0:c0 + nch])
                    h1s = []
                    for h in range(2):
                        ps = psum_mlp.tile([128, cfg.node_chunk], f32,
                                           tag="mlpp")
                        nc.tensor.matmul(
                            out=ps[:, :nch],
                            lhsT=w1_sb[:, l * H2 + h * 128:
                                       l * H2 + (h + 1) * 128],
                            rhs=z[:, :nch], start=True, stop=True)
                        h1 = mlp.tile([128, cfg.node_chunk], f32,
                                      tag=f"h1{h}")
                        nc.scalar.activation(
                            out=h1[:, :nch], in_=ps[:, :nch], func=Act.Relu,
                            bias=b1_sb[:, l * 2 + h: l * 2 + h + 1],
                            scale=1.0)
                        h1s.append(h1)
                    ps2 = psum_mlp.tile([128, cfg.node_chunk], f32,
                                        tag="mlpp")
                    for k in range(2):
                        nc.tensor.matmul(
                            out=ps2[:, :nch],
                            lhsT=w2_sb[:, (l * 2 + k) * HID:
                                       (l * 2 + k + 1) * HID],
                            rhs=h1s[k][:, :nch],
                            start=(k == 0), stop=(k == 1))
                    xnT = mlp.tile([128, cfg.node_chunk], f32, tag="xnT")
                    nc.scalar.activation(
                        out=xnT[:, :nch], in_=ps2[:, :nch], func=Act.Relu,
                        bias=b2_sb[:, l:l + 1], scale=1.0)
                    if l < N_LAYERS - 1:
                        nc.sync.dma_start(out=xTs[l + 1][:, c0:c0 + nch],
                                          in_=xnT[:, :nch])
                    # transpose to node-major (cast to table dtype)
                    nblk = -(-nch // 128)
                    xnm = mlp.tile([128, cfg.node_chunk // 128 * C
                                    + (C if cfg.node_chunk % 128 else 0)],
                                   tdt, tag="xnm")
                    for i in range(nblk):
                        b0 = i * 128
                        bn = min(128, nch - b0)
                        tp = psum_tp.tile([128, 128], f32, tag="tp")
                        nc.tensor.transpose(
                            out=tp[:bn, :],
                            in_=xnT[:, b0:b0 + bn],
                            identity=ident_sb[:])
                        nc.vector.tensor_copy(out=xnm[:bn, i * C:i * C + C],
                                              in_=tp[:bn, :])
                        nc.sync.dma_start(
                            out=xowns[l][c0 + b0: c0 + b0 + bn, :],
                            in_=xnm[:bn, i * C:i * C + C])
                if l < N_LAYERS - 1:
                    nc.gpsimd.collective_compute(
                        "AllGather", Alu.bypass, replica_groups=rg,
                        ins=[xowns[l][:, :].opt()],
                        outs=[xtabs[l + 1][:, :].opt()])

            # ---------- graph pooling ----------
            pps = psum_mlp.tile([128, NG], f32, tag="mlpp")
            for t in range(NPT):
                r0 = t * 128
                rows = min(128, NPC - r0)
                px = gp.tile([128, C], tdt, tag="px")
                if rows < 128:
                    nc.vector.memset(px[:], 0.0)
                nc.sync.dma_start(out=px[:rows, :],
                                  in_=xowns[N_LAYERS - 1][r0:r0 + rows, :])
                st = stp.tile([128, NG], tdt, tag="pst")
                nc.vector.tensor_tensor(
                    out=st[:],
                    in0=psegrel_sb[:, t:t + 1].to_broadcast([128, NG]),
                    in1=iota_g_sb[:],
                    op=Alu.is_equal)
                nc.tensor.matmul(out=pps[:], lhsT=px[:], rhs=st[:],
                                 start=(t == 0), stop=(t == NPT - 1))
            pool_sb = headp.tile([128, NG], f32, tag="pool")
            nc.vector.tensor_copy(out=pool_sb[:], in_=pps[:])
            nc.sync.dma_start(out=pool_in[:, :], in_=pool_sb[:])
            nc.gpsimd.collective_compute(
                "AllReduce", Alu.add, replica_groups=rg,
                ins=[pool_in[:, :].opt()], outs=[pool_out[:, :].opt()])
            pooled = headp.tile([128, NG], f32, tag="pooled")
            nc.sync.dma_start(out=pooled[:], in_=pool_out[:, :])

            # ---------- head ----------
            hps = psum_mlp.tile([128, NG], f32, tag="mlpp")
            nc.tensor.matmul(out=hps[:], lhsT=lin1_sb[:], rhs=pooled[:],
                             start=True, stop=True)
            hT = headp.tile([128, NG], f32, tag="hT")
            nc.scalar.activation(out=hT[:], in_=hps[:], func=Act.Relu,
                                 bias=lin1b_sb[:, 0:1], scale=1.0)
            ngt = -(-NG // 128)
            out_sb = headp.tile([128, ngt * N_OUT], f32, tag="outsb")
            for gt in range(ngt):
                g0 = gt * 128
                gn = min(128, NG - g0)
                lp = psum_tp.tile([128, N_OUT], f32, tag="tp")
                nc.tensor.matmul(out=lp[:gn, :], lhsT=hT[:, g0:g0 + gn],
                                 rhs=lin2_sb[:], start=True, stop=False)
                nc.tensor.matmul(out=lp[:gn, :], lhsT=ones1_sb[:, :gn],
                                 rhs=lin2b_sb[:], start=False, stop=True)
                logits = headp.tile([128, N_OUT], f32, tag="lg")
                nc.vector.tensor_copy(out=logits[:gn, :], in_=lp[:gn, :])
                mx = headp.tile([128, 1], f32, tag="mx")
                nc.vector.tensor_reduce(out=mx[:gn, :], in_=logits[:gn, :],
                                        axis=mybir.AxisListType.X,
                                        op=Alu.max)
                sh = headp.tile([128, N_OUT], f32, tag="sh")
                nc.vector.tensor_scalar(
                    out=sh[:gn, :], in0=logits[:gn, :],
                    scalar1=mx[:gn, 0:1], scalar2=None, op0=Alu.subtract)
                ex = headp.tile([128, N_OUT], f32, tag="ex")
                se = headp.tile([128, 1], f32, tag="se")
                nc.scalar.activation(out=ex[:gn, :], in_=sh[:gn, :],
                                     func=Act.Exp, accum_out=se[:gn, :])
                ls = headp.tile([128, 1], f32, tag="ls")
                nc.scalar.activation(out=ls[:gn, :], in_=se[:gn, :],
                                     func=Act.Ln)
                nc.vector.tensor_scalar(
                    out=out_sb[:gn, gt * N_OUT:(gt + 1) * N_OUT],
                    in0=sh[:gn, :], scalar1=ls[:gn, 0:1], scalar2=None,
                    op0=Alu.subtract)
                nc.sync.dma_start(
                    out=out_d[g0:g0 + gn, :],
                    in_=out_sb[:gn, gt * N_OUT:(gt + 1) * N_OUT])

    nc.compile()
    return nc


# =========================================================================
# entry point
# =========================================================================

def _meta_key(meta, cfg):
    import hashlib
    h = hashlib.sha256()
    h.update(meta["gmax_t"].tobytes())
    h.update(np.asarray(meta["first_w"]).tobytes())
    h.update(repr(meta["chunks"]).encode())
    h.update(repr(meta["eps"]).encode())
    h.update(repr((cfg.n_nodes, cfg.n_graphs, cfg.win, cfg.dwin,
                   cfg.gchunk, cfg.node_chunk, cfg.table_f16)).encode())
    return h.hexdigest()


def _run_program(nc, in_maps, sim=False, trace=False):
    if sim:
        from concourse import bass_interp
        msim = bass_interp.MultiCoreSim(nc, N_CORES)
        for i in range(N_CORES):
            for k, v in in_maps[i].items():
                msim.cores[i].tensor(k)[:] = v
        msim.simulate(check_with_hw=False)
        return [{"out": np.array(msim.cores[i].mem_tensor("out"))}
                for i in range(N_CORES)], None
    from concourse.bass_utils import run_bass_kernel_spmd
    res = run_bass_kernel_spmd(nc, in_maps, list(range(N_CORES)), trace=trace)
    return res.results, res


def kernel_with_cfg(ins, cfg, sim=False, trace=False, full=False):
    shared, per_core, meta = _preprocess(ins, cfg)
    key = _meta_key(meta, cfg)
    nc = _BUILD_CACHE.get(key)
    if nc is None:
        nc = _build_program(meta, cfg)
        _BUILD_CACHE[key] = nc
    in_maps = [dict(shared, **pc) for pc in per_core]
    results, res = _run_program(nc, in_maps, sim=sim, trace=trace)
    out = results[0]["out"].astype(np.float32)
    return (out, res) if full else out


def kernel(**inputs) -> np.ndarray:
    return kernel_with_cfg(inputs, DEFAULT_CFG, sim=False)
